# revision 44
# baseline (speedup 1.0000x reference)
"""Banded (sparse) attention encoder block on 8 Trainium2 NeuronCores.

Problem: nn_NeuralEncoder (B=4, S=2000=100 time patches x 20 space patches,
H=512, 8 heads, D=64, RoPE over time-patch timestamps, banded attention
|t_q - t_k| <= 4 tiled over space patches).

Sharding: 8 cores = 4 batches x 2 head-groups (4 heads each).
Host prep: permute tokens to time-major order (u = t*SP + sp) so the banded
mask becomes a contiguous band of keys; transpose x to xT [H, S]; weights and
tables pre-packed into a few fully-contiguous DMA blobs; the RoPE sin table
carries the rotate-half signs (the rotate permutation itself is unsigned).

Device (one SPMD Bass program, all matmuls bf16 with fp32 PSUM):
  - 8 big input DMAs split across both HWDGE rings, first-needed first
  - q/k projection: 4 matmuls -> psum; DVE multiplies psum by per-patch
    cos/sin tables (broadcast APs); rotate-half via an unsigned PE
    permutation matmul; DVE adds the two halves into qT/kT
  - attn_mask==1 fast path: no mask scaling; the softmax denominator column
    of v is a one-time memset
  - per chunk j (100 keys): v projection (issued 2 chunks early to keep the
    PE stream dense; psum -> bf16 on ACT); 4 heads' scoresT into one 4-bank
    psum tile, ONE exp activation evacuates all of them; band masking only
    multiplies the two staircase side-blocks (GPSIMD) - the center is always
    valid
  - AV strip [q, head, d] accumulated over <=3 chunks with a ones column as
    denominator; reciprocal+normalize on DVE; PE transposes rebuild ctxT;
    output projection + DMA interleaved one 128-row column block per chunk
Host epilogue: sum the two head-group partials, add bo, transpose, un-permute
back to space-major order. Falls back to a numpy reference path if the inputs
don't match the expected mask/timestamp structure.
"""

import numpy as np
import ml_dtypes
from contextlib import ExitStack

import concourse.tile as tile
from concourse import bacc, mybir
from concourse import bass_utils

F32 = mybir.dt.float32
BF16 = mybir.dt.bfloat16

# Static problem configuration (hardcoded, matches the reference).
B, T, SP = 4, 100, 20
S = T * SP                  # 2000
H, NH, D = 512, 8, 64
CF = CB = 4
G = 2                       # head groups (tensor-parallel factor)
HPC = NH // G               # heads per core = 4
HG = HPC * D                # 256 hidden per group
VP = 65                     # per-head v width: 64 dims + denominator column
ROPE_BASE = 10000.0
N_CORES = 8

PPC = 5                     # time patches per key chunk
CK = PPC * SP               # 100 keys per chunk
NCH = T // PPC              # 20 key chunks / query strips
SC = 500                    # free-dim chunk for [128, 500] psum tiles
NSC = S // SC               # 4
MW = 280                    # max scoresT query-window width

_CACHE = {}


def _qwin(j):
    """Token range of the query window covered by scoresT of key chunk j."""
    lo = max(0, PPC * j - PPC) * SP
    hi = min(T, PPC * j + PPC + CF) * SP
    return lo, hi


def _build_program():
    nc = bacc.Bacc("TRN2", target_bir_lowering=False, debug=False,
                   num_devices=N_CORES)

    xT = nc.dram_tensor("xT", [H, S], BF16, kind="ExternalInput").ap()
    wqk = nc.dram_tensor("wqk", [128, 8, HG], BF16,
                         kind="ExternalInput").ap()
    w2 = nc.dram_tensor("w2", [128, 2048], BF16, kind="ExternalInput").ap()
    csb = nc.dram_tensor("csb", [128, 328], BF16, kind="ExternalInput").ap()
    mblob = nc.dram_tensor("mblob", [CK, 280], BF16,
                           kind="ExternalInput").ap()
    outT = nc.dram_tensor("outT", [H, S], BF16, kind="ExternalOutput").ap()

    with ExitStack() as ctx:
        tc = ctx.enter_context(tile.TileContext(nc))
        consts = ctx.enter_context(tc.tile_pool(name="consts", bufs=1))
        work = ctx.enter_context(tc.tile_pool(name="work", bufs=48))
        psum = ctx.enter_context(tc.tile_pool(name="psum", bufs=8,
                                              space="PSUM"))

        # ---- persistent tiles (blob views keep the DMA count tiny) ----
        xt = [consts.tile([128, S], BF16, tag=f"xt{kc}", name=f"xt{kc}")
              for kc in range(4)]
        wqk_sb = consts.tile([128, 8, HG], BF16, tag="wqk")
        w2_sb = consts.tile([128, 2048], BF16, tag="w2")
        csb_sb = consts.tile([128, 328], BF16, tag="csb")
        mb_sb = consts.tile([CK, 280], BF16, tag="mb")
        wq_sb = wqk_sb[:, 0:4]
        wk_sb = wqk_sb[:, 4:8]
        wv_sb = w2_sb[:, 0:1024].rearrange("p (c m) -> p c m", m=HG)
        wo_sb = w2_sb[:, 1024:2048].rearrange("p (c m) -> p c m", m=H)
        cos_sb = csb_sb[:, 0:100]
        sin_sb = csb_sb[:, 100:200]
        p_sb = csb_sb[:, 200:328]
        mL_sb = mb_sb[:, 0:100]
        mR_sb = mb_sb[:, 100:180]
        id_sb = mb_sb[:, 180:280]

        qT = [consts.tile([128, S], BF16, tag=f"qT{hp}", name=f"qT{hp}")
              for hp in range(2)]
        kT = [consts.tile([128, S], BF16, tag=f"kT{hp}", name=f"kT{hp}")
              for hp in range(2)]
        ctx_all = consts.tile([128, 2, S], BF16, tag="ctx", name="ctx")
        v_all = consts.tile([CK, NCH, HPC, VP], BF16, tag="v", name="v")

        # ---- input DMAs: first-needed first, split across both rings ----
        nc.sync.dma_start(out=wqk_sb, in_=wqk)
        nc.sync.dma_start(out=xt[1][:, 0:1000], in_=xT[128:256, 0:1000])
        nc.sync.dma_start(out=xt[0][:, 1000:S], in_=xT[0:128, 1000:S])
        nc.sync.dma_start(out=w2_sb, in_=w2)
        nc.sync.dma_start(out=xt[1][:, 1000:S], in_=xT[128:256, 1000:S])
        nc.scalar.dma_start(out=csb_sb, in_=csb)
        nc.scalar.dma_start(out=xt[0][:, 0:1000], in_=xT[0:128, 0:1000])
        nc.scalar.dma_start(out=xt[2][:, 0:1000], in_=xT[256:384, 0:1000])
        nc.scalar.dma_start(out=xt[3][:, 0:1000], in_=xT[384:512, 0:1000])
        nc.scalar.dma_start(out=mb_sb, in_=mblob)

        # softmax denominator column of v (attn_mask == 1)
        nc.gpsimd.memset(v_all[:, :, :, D], 1.0)

        # PE warmup burst: dependency-free junk matmuls fill the DMA wait
        # and start the HAM activity clock early (values never read)
        junk = consts.tile([128, 512], BF16, tag="junk")
        nc.vector.memset(junk, 0.0)
        for _ in range(12):
            pj = psum.tile([128, SC], F32, tag="pp", bufs=2)
            nc.tensor.matmul(pj[:, 0:SC], lhsT=junk[:, 0:128],
                             rhs=junk[:, 0:SC], start=True, stop=True)

        # ---- q/k projections + RoPE ----
        def qk_proj(w_sb, dst, hp, sc):
            cols = slice(SC * sc, SC * (sc + 1))
            p0 = 25 * sc
            ps = psum.tile([128, SC], F32, tag="pp", bufs=2)
            for kc in range(4):
                nc.tensor.matmul(
                    ps,
                    lhsT=w_sb[:, kc, 128 * hp:128 * (hp + 1)],
                    rhs=xt[kc][:, cols],
                    start=(kc == 0), stop=(kc == 3),
                )
            # one fused DVE op: t12[:,0] = ps*cos, t12[:,1] = ps*sinS
            # (cos and the sign-folded sin are adjacent in the csb blob)
            t12 = work.tile([128, 2, SC], BF16, tag="t12", bufs=3)
            cs2 = csb_sb[:, 0:200].rearrange("p (c t) -> p c t", t=T)
            nc.vector.tensor_mul(
                out=t12.rearrange("p c (a b) -> p c a b", b=SP),
                in0=ps.rearrange("p (a b) -> p a b", b=SP)
                    .unsqueeze(1).broadcast_to([128, 2, 25, SP]),
                in1=cs2[:, :, p0:p0 + 25].unsqueeze(3)
                    .broadcast_to([128, 2, 25, SP]))
            # unsigned rotate-half of the sin part on the PE, then combine;
            # psr borrows the scores psum ring to keep the pp ring shallow
            psr = psum.tile([128, 2, 512], F32, tag="pss", bufs=2)
            nc.tensor.matmul(psr[:, 0, 0:SC], lhsT=p_sb, rhs=t12[:, 1],
                             start=True, stop=True)
            nc.vector.tensor_add(out=dst[:, cols], in0=t12[:, 0],
                                 in1=psr[:, 0, 0:SC])

        # ---- v projection (psum shares the [128, 500] "pp" ring) ----
        def v_proj(vc):
            rows = slice(CK * vc, CK * (vc + 1))
            ps = psum.tile([128, SC], F32, tag="pp", bufs=2)
            for kc in range(4):
                nc.tensor.matmul(
                    ps[0:CK, 0:HG],
                    lhsT=xt[kc][:, rows],
                    rhs=wv_sb[:, kc, :],
                    start=(kc == 0), stop=(kc == 3),
                )
            nc.scalar.copy(
                out=v_all[:, vc, :, 0:D],
                in_=ps[0:CK, 0:HG].rearrange("p (h e) -> p h e", e=D))

        # ---- attention ----
        exp_t = {}
        cs_t = {}

        def scores_chunk(j):
            qlo, qhi = _qwin(j)
            w = qhi - qlo
            et = work.tile([CK, HPC, MW], BF16, tag="et4", bufs=5)
            for hp in range(2):
                # two heads' scores into one double-buffered 2-bank psum
                # pair; one exp activation evacuates both
                ps = psum.tile([128, 2, 512], F32, tag="pss", bufs=2)
                for hh in range(2):
                    hb = 64 * hh
                    nc.tensor.matmul(
                        ps[0:CK, hh, :w],
                        lhsT=kT[hp][hb:hb + 64, CK * j:CK * (j + 1)],
                        rhs=qT[hp][hb:hb + 64, qlo:qhi],
                        start=True, stop=True,
                    )
                h2 = slice(2 * hp, 2 * hp + 2)
                nc.scalar.activation(out=et[:, h2, :w],
                                     in_=ps[0:CK, :, :w],
                                     func=mybir.ActivationFunctionType.Exp,
                                     scale=0.125)
            # band masking on GPSIMD, left blocks first: they gate this
            # iteration's av_mm (its chunk-j matmuls read cols 0:100); the
            # right blocks aren't read until next iteration.
            rlo = 100 if j == 0 else (None if j == NCH - 1 else 200)
            for hp in range(2):
                h2 = slice(2 * hp, 2 * hp + 2)
                if j > 0:
                    nc.gpsimd.tensor_mul(
                        out=et[:, h2, 0:CK], in0=et[:, h2, 0:CK],
                        in1=mL_sb.unsqueeze(1).broadcast_to([CK, 2, CK]))
            for hp in range(2):
                h2 = slice(2 * hp, 2 * hp + 2)
                if rlo is not None:
                    nc.gpsimd.tensor_mul(
                        out=et[:, h2, rlo:rlo + 80],
                        in0=et[:, h2, rlo:rlo + 80],
                        in1=mR_sb.unsqueeze(1).broadcast_to([CK, 2, 80]))
            exp_t[j] = et

        def av_mm(i):
            # chunk i first: it covers the strip fully (start=True sets
            # has_written; the left neighbor accumulates on partitions 0:80)
            chunks = [c for c in (i, i - 1, i + 1) if 0 <= c < NCH]
            ps = psum.tile([CK, HPC, VP], F32, tag="pav", bufs=1)
            for h in range(HPC):
                for n, j in enumerate(chunks):
                    qlo, qhi = _qwin(j)
                    lo_g, hi_g = max(CK * i, qlo), min(CK * i + CK, qhi)
                    nc.tensor.matmul(
                        ps[0:hi_g - lo_g, h, :],
                        lhsT=exp_t[j][:, h, lo_g - qlo:hi_g - qlo],
                        rhs=v_all[:, j, h, :],
                        start=(n == 0), stop=(n == len(chunks) - 1),
                    )
            rcp = work.tile([CK, HPC], F32, tag="rcp", bufs=3)
            nc.vector.reciprocal(out=rcp, in_=ps[:, :, D])
            cs = work.tile([CK, HPC, D], BF16, tag="cs", bufs=3)
            nc.vector.tensor_mul(
                out=cs, in0=ps[:, :, 0:D],
                in1=rcp.unsqueeze(2).broadcast_to([CK, HPC, D]))
            cs_t[i] = cs

        def av_tr(i):
            csf = cs_t.pop(i).rearrange("p h e -> p (h e)")
            pt = psum.tile([128, 2, CK], BF16, tag="ptr", bufs=1)
            for hp in range(2):
                nc.tensor.transpose(pt[:, hp, :],
                                    csf[:, 128 * hp:128 * (hp + 1)], id_sb)
            nc.vector.tensor_copy(out=ctx_all[:, :, CK * i:CK * (i + 1)],
                                  in_=pt)

        # ---- output projection, one 128-row column block at a time ----
        def out_oc(c, oc, lo=0, hi=SC, eng=None):
            w = hi - lo
            cols = slice(SC * c + lo, SC * c + hi)
            ps = psum.tile([128, SC], F32, tag="pp", bufs=2)
            for hp in range(2):
                nc.tensor.matmul(
                    ps[:, :w],
                    lhsT=wo_sb[:, hp, 128 * oc:128 * (oc + 1)],
                    rhs=ctx_all[:, hp, cols],
                    start=(hp == 0), stop=(hp == 1),
                )
            ost = work.tile([128, SC], BF16, tag="ost", bufs=3)
            if eng is nc.vector:
                nc.vector.tensor_copy(out=ost[:, :w], in_=ps[:, :w])
            else:
                nc.scalar.copy(out=ost[:, :w], in_=ps[:, :w])
            nc.sync.dma_start(out=outT[128 * oc:128 * (oc + 1), cols],
                              in_=ost[:, :w])

        # ---- software-pipelined main loop ----
        def qk_call(sc, m):
            hp = m % 2
            if m < 2:
                qk_proj(wq_sb, qT[hp], hp, sc)
            else:
                qk_proj(wk_sb, kT[hp], hp, sc)

        ranges = [0, 4, 9, 14, NCH]
        # v is independent of RoPE and evacuates psum on ACT (qk evacuates
        # on DVE): interleaving them keeps the shared psum ring's releases
        # off the DVE serial path and the PE stream dense from the start
        qk_call(0, 0)
        qk_call(0, 1)
        v_proj(0)
        qk_call(0, 2)
        v_proj(1)
        qk_call(0, 3)
        v_proj(2)
        v_proj(3)
        for sc in range(NSC):
            for idx, j in enumerate(range(ranges[sc], ranges[sc + 1])):
                # scores first; then ~3us of independent PE work hides the
                # exp+mask latency before av_mm needs the masked et tile
                scores_chunk(j)
                if j < 2:
                    # x column tails, issued behind the first exps so the
                    # scalar queue serves compute first
                    kc = 2 + j
                    nc.scalar.dma_start(out=xt[kc][:, 1000:S],
                                        in_=xT[128 * kc:128 * (kc + 1),
                                               1000:S])
                if j >= 2:
                    av_tr(j - 2)
                if sc + 1 < NSC and idx < 4:
                    qk_call(sc + 1, idx)
                if j + 4 < NCH:
                    v_proj(j + 4)
                if j >= 6 and (j - 6) % 5 < 4 and (j - 6) // 5 < 3:
                    out_oc((j - 6) // 5, (j - 6) % 5)
                if j >= 1:
                    av_mm(j - 1)
        # epilogue: drain the pipeline; av_tr(18) + the first 400 columns of
        # the last output block fill the PE while the final strip completes
        av_tr(NCH - 2)

        def out_final(lo, hi):
            # all 4 row-blocks into one staging tile, ONE output DMA
            w = hi - lo
            ost4 = work.tile([128, 4, SC], BF16, tag="ost4", bufs=2)
            if w <= 125:
                # narrow finale: all 4 row-blocks fit one psum bank -> one
                # psum allocation, one cast, shortest possible end chain
                ps = psum.tile([128, SC], F32, tag="pp", bufs=2)
                psv = ps.rearrange("p (c m) -> p c m", m=125)
                for oc in range(4):
                    for hp in range(2):
                        nc.tensor.matmul(
                            psv[:, oc, :w],
                            lhsT=wo_sb[:, hp, 128 * oc:128 * (oc + 1)],
                            rhs=ctx_all[:, hp, SC * 3 + lo:SC * 3 + hi],
                            start=(hp == 0), stop=(hp == 1),
                        )
                nc.scalar.copy(out=ost4[:, :, :w], in_=psv[:, :, :w])
            else:
                for oc in range(4):
                    ps = psum.tile([128, SC], F32, tag="pp", bufs=2)
                    for hp in range(2):
                        nc.tensor.matmul(
                            ps[:, :w],
                            lhsT=wo_sb[:, hp, 128 * oc:128 * (oc + 1)],
                            rhs=ctx_all[:, hp, SC * 3 + lo:SC * 3 + hi],
                            start=(hp == 0), stop=(hp == 1),
                        )
                    if oc % 2:
                        nc.vector.tensor_copy(out=ost4[:, oc, :w],
                                              in_=ps[:, :w])
                    else:
                        nc.scalar.copy(out=ost4[:, oc, :w], in_=ps[:, :w])
            nc.sync.dma_start(
                out=outT.rearrange("(c p) m -> p c m", p=128)
                        [:, :, SC * 3 + lo:SC * 3 + hi],
                in_=ost4[:, :, :w])

        av_mm(NCH - 1)
        out_final(0, 400)
        av_tr(NCH - 1)
        out_final(400, SC)

    nc.finalize()   # Bacc register allocation + DCE before serialization
    return nc


def _get_program():
    if "nc" not in _CACHE:
        _CACHE["nc"] = _build_program()
    return _CACHE["nc"]


def _host_prep(x, Wq, Wk, Wv, Wo):
    """Build the 8 per-core input maps."""
    bf16 = ml_dtypes.bfloat16

    def to_tm(a):
        # [B, S, ...] space-major -> time-major (u = t*SP + sp)
        return (a.reshape(B, SP, T, *a.shape[2:])
                 .swapaxes(1, 2)
                 .reshape(B, S, *a.shape[2:]))

    x_tm = to_tm(np.ascontiguousarray(x))

    # RoPE tables, per time patch; sin carries the rotate-half signs
    inv_freq = 1.0 / (ROPE_BASE ** (np.arange(0, D, 2, dtype=np.float32) / D))
    tt = np.arange(T, dtype=np.float32)
    freqs = tt[:, None] * inv_freq[None, :]
    emb = np.concatenate([freqs, freqs], axis=-1)      # [T, D]
    cos_t = np.cos(emb).astype(np.float32).T           # [64, T]
    sin_t = np.sin(emb).astype(np.float32).T
    sinS = sin_t.copy()
    sinS[D // 2:] *= -1.0                              # sign fold for rotate

    # unsigned rotate-half permutation (sinS already carries the signs)
    p = np.zeros((128, 128), np.float32)
    for blk in (0, 64):
        for d in range(32):
            p[blk + d + 32, blk + d] = 1.0
            p[blk + d, blk + d + 32] = 1.0
    csb = np.concatenate([np.vstack([cos_t, cos_t]),
                          np.vstack([sinS, sinS]), p], axis=1)  # [128, 328]

    # staircase band masks for the side-blocks + transpose identity
    kr = np.arange(CK)[:, None] // SP
    cl = np.arange(CK)[None, :] // SP
    m2L = (cl >= kr + 1).astype(np.float32)            # [100, 100]
    cr = np.arange(80)[None, :] // SP
    m2R = (cr <= kr - 1).astype(np.float32)            # [100, 80]
    mblob = np.concatenate([m2L, m2R, np.eye(CK, dtype=np.float32)],
                           axis=1)                     # [100, 280]

    def pack4(w):       # [512, M] -> [128, 4, M]
        return np.ascontiguousarray(
            w.reshape(4, 128, w.shape[1]).transpose(1, 0, 2))

    in_maps = []
    for c in range(N_CORES):
        b, g = c // 2, c % 2
        hcols = slice(HG * g, HG * (g + 1))
        wqk = np.concatenate([pack4(Wq[:, hcols]), pack4(Wk[:, hcols])],
                             axis=1)                   # [128, 8, 256]
        w2 = np.concatenate(
            [pack4(Wv[:, hcols]).reshape(128, 1024),
             np.ascontiguousarray(
                 Wo[hcols, :].reshape(2, 128, H).transpose(1, 0, 2)
             ).reshape(128, 1024)], axis=1)            # [128, 2048]
        in_maps.append({
            "xT": np.ascontiguousarray(x_tm[b].T).astype(bf16),
            "wqk": wqk.astype(bf16),
            "w2": w2.astype(bf16),
            "csb": csb.astype(bf16),
            "mblob": mblob.astype(bf16),
        })
    return in_maps


def _numpy_fallback(x, attn_mask, timestamps, Wq, bq, Wk, bk, Wv, bv, Wo, bo):
    """Reference-equivalent numpy path for unexpected input structure."""
    inv_freq = 1.0 / (ROPE_BASE ** (np.arange(0, D, 2, dtype=np.float32) / D))
    tt = np.arange(T, dtype=np.float32)
    emb = np.concatenate([tt[:, None] * inv_freq[None, :]] * 2, axis=-1)
    cos_t, sin_t = np.cos(emb), np.sin(emb)

    def heads(w, b):
        return (x @ w + b).reshape(B, S, NH, D).transpose(0, 2, 1, 3)
    q, k, v = heads(Wq, bq), heads(Wk, bk), heads(Wv, bv)
    cos = cos_t[timestamps][:, None]
    sin = sin_t[timestamps][:, None]

    def rot(u):
        return np.concatenate((-u[..., D // 2:], u[..., :D // 2]), axis=-1)
    q = q * cos + rot(q) * sin
    k = k * cos + rot(k) * sin
    scores = np.einsum('bhqd,bhkd->bhqk', q, k) / np.sqrt(np.float32(D))
    ones = np.ones((T, T), np.float32)
    m = np.triu(ones, k=-CF).T * np.triu(ones, k=-CB)
    m = np.tile(m, (SP, SP))
    mask = (m[None, None] * attn_mask[:, None, None, :]) > 0
    scores = np.where(mask, scores, -1e9)
    scores -= scores.max(axis=-1, keepdims=True)
    e = np.exp(scores)
    attn = e / e.sum(axis=-1, keepdims=True)
    out = np.einsum('bhqk,bhkd->bhqd', attn, v)
    out = out.transpose(0, 2, 1, 3).reshape(B, S, H)
    return (out @ Wo + bo).astype(np.float32)


def kernel(x, attn_mask, timestamps, Wq, bq, Wk, bk, Wv, bv, Wo, bo,
           **_ignored):
    x = np.asarray(x, np.float32)
    attn_mask = np.asarray(attn_mask)
    timestamps = np.asarray(timestamps)
    Wq, Wk, Wv, Wo = (np.asarray(a, np.float32) for a in (Wq, Wk, Wv, Wo))
    bq, bk, bv, bo = (np.asarray(a, np.float32) for a in (bq, bk, bv, bo))

    # the device program bakes in the time-patch structure, an all-ones
    # attn_mask, and zero qkv biases; anything else takes the numpy path
    ts_tm = (timestamps.reshape(B, SP, T).swapaxes(1, 2).reshape(B, S))
    expect_ts = np.broadcast_to(
        np.repeat(np.arange(T, dtype=ts_tm.dtype), SP), (B, S))
    if (not np.array_equal(ts_tm, expect_ts)
            or not np.all(attn_mask == 1)
            or np.any(bq) or np.any(bk) or np.any(bv)):
        return _numpy_fallback(x, attn_mask, timestamps,
                               Wq, bq, Wk, bk, Wv, bv, Wo, bo)

    nc = _get_program()
    in_maps = _host_prep(x, Wq, Wk, Wv, Wo)

    res = bass_utils.run_bass_kernel_spmd(nc, in_maps,
                                          core_ids=list(range(N_CORES)))
    _CACHE["last_results"] = res

    out = np.empty((B, S, H), np.float32)
    for b in range(B):
        o = (res.results[2 * b]["outT"].astype(np.float32) +
             res.results[2 * b + 1]["outT"].astype(np.float32))
        o_tm = o.T + bo[None, :]                        # [2000, 512]
        out[b] = (o_tm.reshape(T, SP, H)
                      .swapaxes(0, 1)
                      .reshape(S, H))
    return out


# revision 45
# speedup vs baseline: 1.0613x; 1.0613x over previous
"""Banded (sparse) attention encoder block on 8 Trainium2 NeuronCores.

Problem: nn_NeuralEncoder (B=4, S=2000=100 time patches x 20 space patches,
H=512, 8 heads, D=64, RoPE over time-patch timestamps, banded attention
|t_q - t_k| <= 4 tiled over space patches).

Sharding: 8 cores = 4 batches x 2 head-groups (4 heads each).
Host prep: permute tokens to time-major order (u = t*SP + sp) so the banded
mask becomes a contiguous band of keys; transpose x to xT [H, S]; weights and
tables pre-packed into a few fully-contiguous DMA blobs; the RoPE sin table
carries the rotate-half signs (the rotate permutation itself is unsigned).

Device (one SPMD Bass program, all matmuls bf16 with fp32 PSUM):
  - 8 big input DMAs split across both HWDGE rings, first-needed first
  - q/k projection: 4 matmuls -> psum; DVE multiplies psum by per-patch
    cos/sin tables (broadcast APs); rotate-half via an unsigned PE
    permutation matmul; DVE adds the two halves into qT/kT
  - attn_mask==1 fast path: no mask scaling; the softmax denominator column
    of v is a one-time memset
  - per chunk j (100 keys): v projection (issued 2 chunks early to keep the
    PE stream dense; psum -> bf16 on ACT); 4 heads' scoresT into one 4-bank
    psum tile, ONE exp activation evacuates all of them; band masking only
    multiplies the two staircase side-blocks (GPSIMD) - the center is always
    valid
  - AV strip [q, head, d] accumulated over <=3 chunks with a ones column as
    denominator; reciprocal+normalize on DVE; PE transposes rebuild ctxT;
    output projection + DMA interleaved one 128-row column block per chunk
Host epilogue: sum the two head-group partials, add bo, transpose, un-permute
back to space-major order. Falls back to a numpy reference path if the inputs
don't match the expected mask/timestamp structure.
"""

import numpy as np
import ml_dtypes
from contextlib import ExitStack

import concourse.tile as tile
from concourse import bacc, mybir
from concourse import bass_utils

F32 = mybir.dt.float32
BF16 = mybir.dt.bfloat16

# Static problem configuration (hardcoded, matches the reference).
B, T, SP = 4, 100, 20
S = T * SP                  # 2000
H, NH, D = 512, 8, 64
CF = CB = 4
G = 2                       # head groups (tensor-parallel factor)
HPC = NH // G               # heads per core = 4
HG = HPC * D                # 256 hidden per group
VP = 65                     # per-head v width: 64 dims + denominator column
ROPE_BASE = 10000.0
N_CORES = 8

PPC = 5                     # time patches per key chunk
CK = PPC * SP               # 100 keys per chunk
NCH = T // PPC              # 20 key chunks / query strips
SC = 500                    # free-dim chunk for [128, 500] psum tiles
NSC = S // SC               # 4
MW = 280                    # max scoresT query-window width

_CACHE = {}


def _qwin(j):
    """Token range of the query window covered by scoresT of key chunk j."""
    lo = max(0, PPC * j - PPC) * SP
    hi = min(T, PPC * j + PPC + CF) * SP
    return lo, hi


def _build_program():
    nc = bacc.Bacc("TRN2", target_bir_lowering=False, debug=False,
                   num_devices=N_CORES)

    xT = nc.dram_tensor("xT", [H, S], BF16, kind="ExternalInput").ap()
    wqk = nc.dram_tensor("wqk", [128, 8, HG], BF16,
                         kind="ExternalInput").ap()
    w2 = nc.dram_tensor("w2", [128, 2048], BF16, kind="ExternalInput").ap()
    csb = nc.dram_tensor("csb", [128, 328], BF16, kind="ExternalInput").ap()
    mblob = nc.dram_tensor("mblob", [CK, 280], BF16,
                           kind="ExternalInput").ap()
    outT = nc.dram_tensor("outT", [H, S], BF16, kind="ExternalOutput").ap()

    with ExitStack() as ctx:
        tc = ctx.enter_context(tile.TileContext(nc))
        consts = ctx.enter_context(tc.tile_pool(name="consts", bufs=1))
        work = ctx.enter_context(tc.tile_pool(name="work", bufs=48))
        psum = ctx.enter_context(tc.tile_pool(name="psum", bufs=8,
                                              space="PSUM"))

        # ---- persistent tiles (blob views keep the DMA count tiny) ----
        xt = [consts.tile([128, S], BF16, tag=f"xt{kc}", name=f"xt{kc}")
              for kc in range(4)]
        wqk_sb = consts.tile([128, 8, HG], BF16, tag="wqk")
        w2_sb = consts.tile([128, 2048], BF16, tag="w2")
        csb_sb = consts.tile([128, 328], BF16, tag="csb")
        mb_sb = consts.tile([CK, 280], BF16, tag="mb")
        wq_sb = wqk_sb[:, 0:4]
        wk_sb = wqk_sb[:, 4:8]
        wv_sb = w2_sb[:, 0:1024].rearrange("p (c m) -> p c m", m=HG)
        wo_sb = w2_sb[:, 1024:2048].rearrange("p (c m) -> p c m", m=H)
        cos_sb = csb_sb[:, 0:100]
        sin_sb = csb_sb[:, 100:200]
        p_sb = csb_sb[:, 200:328]
        mL_sb = mb_sb[:, 0:100]
        mR_sb = mb_sb[:, 100:180]
        id_sb = mb_sb[:, 180:280]

        qT = [consts.tile([128, S], BF16, tag=f"qT{hp}", name=f"qT{hp}")
              for hp in range(2)]
        kT = [consts.tile([128, S], BF16, tag=f"kT{hp}", name=f"kT{hp}")
              for hp in range(2)]
        ctx_all = consts.tile([128, 2, S], BF16, tag="ctx", name="ctx")
        v_all = consts.tile([CK, NCH, HPC, VP], BF16, tag="v", name="v")

        # ---- input DMAs: first-needed first, split across both rings ----
        nc.sync.dma_start(out=wqk_sb, in_=wqk)
        nc.sync.dma_start(out=xt[1][:, 0:1000], in_=xT[128:256, 0:1000])
        nc.sync.dma_start(out=xt[0][:, 1000:S], in_=xT[0:128, 1000:S])
        nc.sync.dma_start(out=w2_sb, in_=w2)
        nc.sync.dma_start(out=xt[1][:, 1000:S], in_=xT[128:256, 1000:S])
        nc.scalar.dma_start(out=csb_sb, in_=csb)
        nc.scalar.dma_start(out=xt[0][:, 0:1000], in_=xT[0:128, 0:1000])
        nc.scalar.dma_start(out=xt[2][:, 0:1000], in_=xT[256:384, 0:1000])
        nc.scalar.dma_start(out=xt[3][:, 0:1000], in_=xT[384:512, 0:1000])
        nc.scalar.dma_start(out=mb_sb, in_=mblob)

        # softmax denominator column of v (attn_mask == 1)
        nc.gpsimd.memset(v_all[:, :, :, D], 1.0)

        # PE warmup burst: dependency-free junk matmuls fill the DMA wait
        # and start the HAM activity clock early (values never read)
        junk = consts.tile([128, 512], BF16, tag="junk")
        nc.vector.memset(junk, 0.0)
        for _ in range(12):
            pj = psum.tile([128, SC], F32, tag="pp", bufs=2)
            nc.tensor.matmul(pj[:, 0:SC], lhsT=junk[:, 0:128],
                             rhs=junk[:, 0:SC], start=True, stop=True)

        # ---- q/k projections + RoPE ----
        def qk_proj(w_sb, dst, hp, sc):
            cols = slice(SC * sc, SC * (sc + 1))
            p0 = 25 * sc
            ps = psum.tile([128, SC], F32, tag="pp", bufs=2)
            for kc in range(4):
                nc.tensor.matmul(
                    ps,
                    lhsT=w_sb[:, kc, 128 * hp:128 * (hp + 1)],
                    rhs=xt[kc][:, cols],
                    start=(kc == 0), stop=(kc == 3),
                )
            # one fused DVE op: t12[:,0] = ps*cos, t12[:,1] = ps*sinS
            # (cos and the sign-folded sin are adjacent in the csb blob)
            t12 = work.tile([128, 2, SC], BF16, tag="t12", bufs=4)
            cs2 = csb_sb[:, 0:200].rearrange("p (c t) -> p c t", t=T)
            nc.vector.tensor_mul(
                out=t12.rearrange("p c (a b) -> p c a b", b=SP),
                in0=ps.rearrange("p (a b) -> p a b", b=SP)
                    .unsqueeze(1).broadcast_to([128, 2, 25, SP]),
                in1=cs2[:, :, p0:p0 + 25].unsqueeze(3)
                    .broadcast_to([128, 2, 25, SP]))
            # unsigned rotate-half of the sin part on the PE, then combine;
            # psr borrows the scores psum ring to keep the pp ring shallow
            psr = psum.tile([128, 2, 512], F32, tag="pss", bufs=2)
            nc.tensor.matmul(psr[:, 0, 0:SC], lhsT=p_sb, rhs=t12[:, 1],
                             start=True, stop=True)
            nc.vector.tensor_add(out=dst[:, cols], in0=t12[:, 0],
                                 in1=psr[:, 0, 0:SC])

        # ---- v projection (psum shares the [128, 500] "pp" ring) ----
        def v_proj(vc):
            rows = slice(CK * vc, CK * (vc + 1))
            ps = psum.tile([128, SC], F32, tag="pp", bufs=2)
            for kc in range(4):
                nc.tensor.matmul(
                    ps[0:CK, 0:HG],
                    lhsT=xt[kc][:, rows],
                    rhs=wv_sb[:, kc, :],
                    start=(kc == 0), stop=(kc == 3),
                )
            nc.scalar.copy(
                out=v_all[:, vc, :, 0:D],
                in_=ps[0:CK, 0:HG].rearrange("p (h e) -> p h e", e=D))

        # ---- attention ----
        exp_t = {}
        cs_t = {}

        def scores_chunk(j):
            qlo, qhi = _qwin(j)
            w = qhi - qlo
            et = work.tile([CK, HPC, MW], BF16, tag="et4", bufs=6)
            for hp in range(2):
                # two heads' scores into one double-buffered 2-bank psum
                # pair; one exp activation evacuates both
                ps = psum.tile([128, 2, 512], F32, tag="pss", bufs=2)
                for hh in range(2):
                    hb = 64 * hh
                    nc.tensor.matmul(
                        ps[0:CK, hh, :w],
                        lhsT=kT[hp][hb:hb + 64, CK * j:CK * (j + 1)],
                        rhs=qT[hp][hb:hb + 64, qlo:qhi],
                        start=True, stop=True,
                    )
                h2 = slice(2 * hp, 2 * hp + 2)
                nc.scalar.activation(out=et[:, h2, :w],
                                     in_=ps[0:CK, :, :w],
                                     func=mybir.ActivationFunctionType.Exp,
                                     scale=0.125)
            # band masking on GPSIMD, left blocks first: they gate this
            # iteration's av_mm (its chunk-j matmuls read cols 0:100); the
            # right blocks aren't read until next iteration.
            rlo = 100 if j == 0 else (None if j == NCH - 1 else 200)
            for hp in range(2):
                h2 = slice(2 * hp, 2 * hp + 2)
                if j > 0:
                    nc.gpsimd.tensor_mul(
                        out=et[:, h2, 0:CK], in0=et[:, h2, 0:CK],
                        in1=mL_sb.unsqueeze(1).broadcast_to([CK, 2, CK]))
            for hp in range(2):
                h2 = slice(2 * hp, 2 * hp + 2)
                if rlo is not None:
                    nc.gpsimd.tensor_mul(
                        out=et[:, h2, rlo:rlo + 80],
                        in0=et[:, h2, rlo:rlo + 80],
                        in1=mR_sb.unsqueeze(1).broadcast_to([CK, 2, 80]))
            exp_t[j] = et

        def av_mm(i):
            # chunk i first: it covers the strip fully (start=True sets
            # has_written; the left neighbor accumulates on partitions 0:80)
            chunks = [c for c in (i, i - 1, i + 1) if 0 <= c < NCH]
            ps = psum.tile([CK, HPC, VP], F32, tag="pav", bufs=1)
            for h in range(HPC):
                for n, j in enumerate(chunks):
                    qlo, qhi = _qwin(j)
                    lo_g, hi_g = max(CK * i, qlo), min(CK * i + CK, qhi)
                    nc.tensor.matmul(
                        ps[0:hi_g - lo_g, h, :],
                        lhsT=exp_t[j][:, h, lo_g - qlo:hi_g - qlo],
                        rhs=v_all[:, j, h, :],
                        start=(n == 0), stop=(n == len(chunks) - 1),
                    )
            rcp = work.tile([CK, HPC], F32, tag="rcp", bufs=3)
            nc.vector.reciprocal(out=rcp, in_=ps[:, :, D])
            cs = work.tile([CK, HPC, D], BF16, tag="cs", bufs=3)
            nc.vector.tensor_mul(
                out=cs, in0=ps[:, :, 0:D],
                in1=rcp.unsqueeze(2).broadcast_to([CK, HPC, D]))
            cs_t[i] = cs

        def av_tr(i):
            csf = cs_t.pop(i).rearrange("p h e -> p (h e)")
            pt = psum.tile([128, 2, CK], BF16, tag="ptr", bufs=1)
            for hp in range(2):
                nc.tensor.transpose(pt[:, hp, :],
                                    csf[:, 128 * hp:128 * (hp + 1)], id_sb)
            nc.vector.tensor_copy(out=ctx_all[:, :, CK * i:CK * (i + 1)],
                                  in_=pt)

        # ---- output projection, one 128-row column block at a time ----
        def out_oc(c, oc, lo=0, hi=SC, eng=None):
            w = hi - lo
            cols = slice(SC * c + lo, SC * c + hi)
            ps = psum.tile([128, SC], F32, tag="pp", bufs=2)
            for hp in range(2):
                nc.tensor.matmul(
                    ps[:, :w],
                    lhsT=wo_sb[:, hp, 128 * oc:128 * (oc + 1)],
                    rhs=ctx_all[:, hp, cols],
                    start=(hp == 0), stop=(hp == 1),
                )
            ost = work.tile([128, SC], BF16, tag="ost", bufs=4)
            if eng is nc.vector:
                nc.vector.tensor_copy(out=ost[:, :w], in_=ps[:, :w])
            else:
                nc.scalar.copy(out=ost[:, :w], in_=ps[:, :w])
            nc.sync.dma_start(out=outT[128 * oc:128 * (oc + 1), cols],
                              in_=ost[:, :w])

        # ---- software-pipelined main loop ----
        def qk_call(sc, m):
            hp = m % 2
            if m < 2:
                qk_proj(wq_sb, qT[hp], hp, sc)
            else:
                qk_proj(wk_sb, kT[hp], hp, sc)

        ranges = [0, 4, 9, 14, NCH]
        # v is independent of RoPE and evacuates psum on ACT (qk evacuates
        # on DVE): interleaving them keeps the shared psum ring's releases
        # off the DVE serial path and the PE stream dense from the start
        qk_call(0, 0)
        qk_call(0, 1)
        v_proj(0)
        qk_call(0, 2)
        v_proj(1)
        qk_call(0, 3)
        v_proj(2)
        v_proj(3)
        for sc in range(NSC):
            for idx, j in enumerate(range(ranges[sc], ranges[sc + 1])):
                # scores first; then ~3us of independent PE work hides the
                # exp+mask latency before av_mm needs the masked et tile
                scores_chunk(j)
                if j < 2:
                    # x column tails, issued behind the first exps so the
                    # scalar queue serves compute first
                    kc = 2 + j
                    nc.scalar.dma_start(out=xt[kc][:, 1000:S],
                                        in_=xT[128 * kc:128 * (kc + 1),
                                               1000:S])
                if j >= 2:
                    av_tr(j - 2)
                if sc + 1 < NSC and idx < 4:
                    qk_call(sc + 1, idx)
                if j + 4 < NCH:
                    v_proj(j + 4)
                if j >= 6 and (j - 6) % 5 < 4 and (j - 6) // 5 < 3:
                    out_oc((j - 6) // 5, (j - 6) % 5)
                if j >= 1:
                    av_mm(j - 1)
        # epilogue: drain the pipeline; av_tr(18) + the first 400 columns of
        # the last output block fill the PE while the final strip completes
        av_tr(NCH - 2)

        def out_final(lo, hi):
            # all 4 row-blocks into one staging tile, ONE output DMA
            w = hi - lo
            ost4 = work.tile([128, 4, SC], BF16, tag="ost4", bufs=2)
            if w <= 125:
                # narrow finale: all 4 row-blocks fit one psum bank -> one
                # psum allocation, one cast, shortest possible end chain
                ps = psum.tile([128, SC], F32, tag="pp", bufs=2)
                psv = ps.rearrange("p (c m) -> p c m", m=125)
                for oc in range(4):
                    for hp in range(2):
                        nc.tensor.matmul(
                            psv[:, oc, :w],
                            lhsT=wo_sb[:, hp, 128 * oc:128 * (oc + 1)],
                            rhs=ctx_all[:, hp, SC * 3 + lo:SC * 3 + hi],
                            start=(hp == 0), stop=(hp == 1),
                        )
                nc.scalar.copy(out=ost4[:, :, :w], in_=psv[:, :, :w])
            else:
                for oc in range(4):
                    ps = psum.tile([128, SC], F32, tag="pp", bufs=2)
                    for hp in range(2):
                        nc.tensor.matmul(
                            ps[:, :w],
                            lhsT=wo_sb[:, hp, 128 * oc:128 * (oc + 1)],
                            rhs=ctx_all[:, hp, SC * 3 + lo:SC * 3 + hi],
                            start=(hp == 0), stop=(hp == 1),
                        )
                    if oc % 2:
                        nc.vector.tensor_copy(out=ost4[:, oc, :w],
                                              in_=ps[:, :w])
                    else:
                        nc.scalar.copy(out=ost4[:, oc, :w], in_=ps[:, :w])
            nc.sync.dma_start(
                out=outT.rearrange("(c p) m -> p c m", p=128)
                        [:, :, SC * 3 + lo:SC * 3 + hi],
                in_=ost4[:, :, :w])

        av_mm(NCH - 1)
        out_final(0, 400)
        av_tr(NCH - 1)
        out_final(400, SC)

    nc.finalize()   # Bacc register allocation + DCE before serialization
    return nc


def _get_program():
    if "nc" not in _CACHE:
        _CACHE["nc"] = _build_program()
    return _CACHE["nc"]


def _host_prep(x, Wq, Wk, Wv, Wo):
    """Build the 8 per-core input maps."""
    bf16 = ml_dtypes.bfloat16

    def to_tm(a):
        # [B, S, ...] space-major -> time-major (u = t*SP + sp)
        return (a.reshape(B, SP, T, *a.shape[2:])
                 .swapaxes(1, 2)
                 .reshape(B, S, *a.shape[2:]))

    x_tm = to_tm(np.ascontiguousarray(x))

    # RoPE tables, per time patch; sin carries the rotate-half signs
    inv_freq = 1.0 / (ROPE_BASE ** (np.arange(0, D, 2, dtype=np.float32) / D))
    tt = np.arange(T, dtype=np.float32)
    freqs = tt[:, None] * inv_freq[None, :]
    emb = np.concatenate([freqs, freqs], axis=-1)      # [T, D]
    cos_t = np.cos(emb).astype(np.float32).T           # [64, T]
    sin_t = np.sin(emb).astype(np.float32).T
    sinS = sin_t.copy()
    sinS[D // 2:] *= -1.0                              # sign fold for rotate

    # unsigned rotate-half permutation (sinS already carries the signs)
    p = np.zeros((128, 128), np.float32)
    for blk in (0, 64):
        for d in range(32):
            p[blk + d + 32, blk + d] = 1.0
            p[blk + d, blk + d + 32] = 1.0
    csb = np.concatenate([np.vstack([cos_t, cos_t]),
                          np.vstack([sinS, sinS]), p], axis=1)  # [128, 328]

    # staircase band masks for the side-blocks + transpose identity
    kr = np.arange(CK)[:, None] // SP
    cl = np.arange(CK)[None, :] // SP
    m2L = (cl >= kr + 1).astype(np.float32)            # [100, 100]
    cr = np.arange(80)[None, :] // SP
    m2R = (cr <= kr - 1).astype(np.float32)            # [100, 80]
    mblob = np.concatenate([m2L, m2R, np.eye(CK, dtype=np.float32)],
                           axis=1)                     # [100, 280]

    def pack4(w):       # [512, M] -> [128, 4, M]
        return np.ascontiguousarray(
            w.reshape(4, 128, w.shape[1]).transpose(1, 0, 2))

    in_maps = []
    for c in range(N_CORES):
        b, g = c // 2, c % 2
        hcols = slice(HG * g, HG * (g + 1))
        wqk = np.concatenate([pack4(Wq[:, hcols]), pack4(Wk[:, hcols])],
                             axis=1)                   # [128, 8, 256]
        w2 = np.concatenate(
            [pack4(Wv[:, hcols]).reshape(128, 1024),
             np.ascontiguousarray(
                 Wo[hcols, :].reshape(2, 128, H).transpose(1, 0, 2)
             ).reshape(128, 1024)], axis=1)            # [128, 2048]
        in_maps.append({
            "xT": np.ascontiguousarray(x_tm[b].T).astype(bf16),
            "wqk": wqk.astype(bf16),
            "w2": w2.astype(bf16),
            "csb": csb.astype(bf16),
            "mblob": mblob.astype(bf16),
        })
    return in_maps


def _numpy_fallback(x, attn_mask, timestamps, Wq, bq, Wk, bk, Wv, bv, Wo, bo):
    """Reference-equivalent numpy path for unexpected input structure."""
    inv_freq = 1.0 / (ROPE_BASE ** (np.arange(0, D, 2, dtype=np.float32) / D))
    tt = np.arange(T, dtype=np.float32)
    emb = np.concatenate([tt[:, None] * inv_freq[None, :]] * 2, axis=-1)
    cos_t, sin_t = np.cos(emb), np.sin(emb)

    def heads(w, b):
        return (x @ w + b).reshape(B, S, NH, D).transpose(0, 2, 1, 3)
    q, k, v = heads(Wq, bq), heads(Wk, bk), heads(Wv, bv)
    cos = cos_t[timestamps][:, None]
    sin = sin_t[timestamps][:, None]

    def rot(u):
        return np.concatenate((-u[..., D // 2:], u[..., :D // 2]), axis=-1)
    q = q * cos + rot(q) * sin
    k = k * cos + rot(k) * sin
    scores = np.einsum('bhqd,bhkd->bhqk', q, k) / np.sqrt(np.float32(D))
    ones = np.ones((T, T), np.float32)
    m = np.triu(ones, k=-CF).T * np.triu(ones, k=-CB)
    m = np.tile(m, (SP, SP))
    mask = (m[None, None] * attn_mask[:, None, None, :]) > 0
    scores = np.where(mask, scores, -1e9)
    scores -= scores.max(axis=-1, keepdims=True)
    e = np.exp(scores)
    attn = e / e.sum(axis=-1, keepdims=True)
    out = np.einsum('bhqk,bhkd->bhqd', attn, v)
    out = out.transpose(0, 2, 1, 3).reshape(B, S, H)
    return (out @ Wo + bo).astype(np.float32)


def kernel(x, attn_mask, timestamps, Wq, bq, Wk, bk, Wv, bv, Wo, bo,
           **_ignored):
    x = np.asarray(x, np.float32)
    attn_mask = np.asarray(attn_mask)
    timestamps = np.asarray(timestamps)
    Wq, Wk, Wv, Wo = (np.asarray(a, np.float32) for a in (Wq, Wk, Wv, Wo))
    bq, bk, bv, bo = (np.asarray(a, np.float32) for a in (bq, bk, bv, bo))

    # the device program bakes in the time-patch structure, an all-ones
    # attn_mask, and zero qkv biases; anything else takes the numpy path
    ts_tm = (timestamps.reshape(B, SP, T).swapaxes(1, 2).reshape(B, S))
    expect_ts = np.broadcast_to(
        np.repeat(np.arange(T, dtype=ts_tm.dtype), SP), (B, S))
    if (not np.array_equal(ts_tm, expect_ts)
            or not np.all(attn_mask == 1)
            or np.any(bq) or np.any(bk) or np.any(bv)):
        return _numpy_fallback(x, attn_mask, timestamps,
                               Wq, bq, Wk, bk, Wv, bv, Wo, bo)

    nc = _get_program()
    in_maps = _host_prep(x, Wq, Wk, Wv, Wo)

    res = bass_utils.run_bass_kernel_spmd(nc, in_maps,
                                          core_ids=list(range(N_CORES)))
    _CACHE["last_results"] = res

    out = np.empty((B, S, H), np.float32)
    for b in range(B):
        o = (res.results[2 * b]["outT"].astype(np.float32) +
             res.results[2 * b + 1]["outT"].astype(np.float32))
        o_tm = o.T + bo[None, :]                        # [2000, 512]
        out[b] = (o_tm.reshape(T, SP, H)
                      .swapaxes(0, 1)
                      .reshape(S, H))
    return out
